# revision 29
# baseline (speedup 1.0000x reference)
"""Trainium2 Bass kernel for nn_MetaSignatureEncoder (GCN encoder with FiLM
signature conditioning), distributed over 8 NeuronCores.

Strategy (graph/data parallel, per the sharding hint):
  - Nodes are padded to NPAD = 8*49*128 = 50176 and sharded contiguously
    across the 8 cores (6272 nodes/core, 49 dst tiles of 128).
  - GCN linearity is exploited twice: norm[e] = dinv[src]*dinv[dst] factors
    out of the message sum, and (A x) @ W == A (x @ W), so pass 1 aggregates
    RAW x rows (256ch bf16, staged replicated to every core's HBM - no
    device-side table build, no collective) and applies [Wsig|W1] to the
    128-node aggregate tile afterwards.  dinv[src] rides in the one-hot
    scatter matrix; dinv[dst] is applied to the aggregate.
  - Pass 1 (edges): per dst-tile of 128 nodes, dma_gather fetches the x rows
    of all in-edges.  Gather calls are sized to the REAL per-(tile,segment)
    edge counts (max across cores, rounded to 128, 3 src-range segments for
    int16 indices, chunked <=1024 - the HW limit).  4 SWDGE queues
    round-robin.  The segment-sum runs on the TensorEngine: for each chunk
    of 128 messages an S matrix (S[j,d] = dinv[src_j] if dst_j == d, built
    by DVE is_equal + multiply) matmuls the gathered rows into PSUM.
  - The graph signature s = sum_n relu(z @ Wsig + b) is reduced over nodes
    with a mask-vector matmul (PE) and AllReduce'd in f32.
  - gamma/beta FiLM vectors are computed redundantly on every core.
  - Encoder: FiLM + relu + LN per tile, conv2 matmul via PE transpose,
    dinv scale, AllGather of the [NPAD, 128] bf16 t' table, second edge
    pass (structure as pass 1 but with pre-scaled 128ch rows and plain
    one-hot S), FiLM + LN epilogue, output.

kernel(**inputs) takes the FULL problem inputs and returns the FULL output.
"""
import sys
import numpy as np
import ml_dtypes

sys.path.insert(0, "/opt/trn_rl_repo")

from concourse import bass, bacc, tile, mybir
from concourse import bass_utils

BF16 = ml_dtypes.bfloat16
dt = mybir.dt

MAX_CALL = 1024          # HW limit on num_idxs per dma_gather
NQ = 4                   # SWDGE queues

# ---------------------------------------------------------------- config ----


class Cfg:
    def __init__(self, NT=49, n_real=50000):
        self.NC = 8           # cores
        self.TP = 128         # partitions / dst-tile size
        self.NT = NT          # dst tiles per core
        self.SPLITS = 3       # src-space segments (int16 idx range)
        self.IN_CH = 256
        self.HID = 256
        self.OUT = 128
        self.FUSED = self.HID + self.HID   # sig(256) | conv1(256)
        self.KX = self.IN_CH // 128        # K chunks for x matmuls
        self.KH = self.HID // 128          # K chunks for conv2 matmul
        self.KA = 3                        # K chunks for augmented fc matmuls
        self.SHARD = self.NT * self.TP
        self.NPAD = self.NC * self.SHARD
        self.BOUNDS = [self.NPAD * k // self.SPLITS
                       for k in range(self.SPLITS)] + [self.NPAD]
        self.n_real = n_real
        self.LN_EPS = 1e-5


FULL = Cfg()

# ------------------------------------------------------------ host side -----


def _wrap16(vals, nrows=128):
    """dma_gather index layout: idx j at [j % 16, j // 16], replicated to all
    8 q7 core groups (rows 16k+p == row p)."""
    n = vals.shape[0]
    assert n % 16 == 0
    w = vals.reshape(n // 16, 16).T          # [16, n/16]
    return np.tile(w, (nrows // 16, 1))      # [128, n/16]


def _pmaj(vals, TP=128):
    """[NT*TP] -> [TP, NT] partition-major (tile t col, partition p row)."""
    return np.ascontiguousarray(vals.reshape(-1, TP).T)


def preprocess(edge_index, cfg):
    """Integer-only graph preprocessing -> call plan + per-core indices.

    Returns (deg, calls, idx_cols, seg_cols, per_core); calls is the SHARED
    plan [(t, k, slots, idx_off, seg_off)] with slots = max real count over
    cores rounded to 128, balanced-chunked <= MAX_CALL.
    """
    src = np.asarray(edge_index[0], dtype=np.int64)
    dst = np.asarray(edge_index[1], dtype=np.int64)
    deg = np.bincount(src, minlength=cfg.NPAD).astype(np.float32) + 1.0
    dinv = deg ** -0.5

    SP = cfg.SPLITS
    lists = [[[None] * SP for _ in range(cfg.NT)] for _ in range(cfg.NC)]
    shard_of = dst // cfg.SHARD
    for c in range(cfg.NC):
        m = shard_of == c
        s_c = src[m]
        d_c = dst[m] - c * cfg.SHARD
        tile_of = d_c // cfg.TP
        order = np.argsort(tile_of, kind="stable")
        s_c, d_c, tile_of = s_c[order], d_c[order], tile_of[order]
        bounds = np.searchsorted(tile_of, np.arange(cfg.NT + 1))
        for t in range(cfg.NT):
            sl = slice(bounds[t], bounds[t + 1])
            s_t = s_c[sl]
            d_t = d_c[sl] - t * cfg.TP
            for k in range(SP):
                a = (s_t >= cfg.BOUNDS[k]) & (s_t < cfg.BOUNDS[k + 1])
                o = np.argsort(s_t[a], kind="stable")  # src order: locality
                lists[c][t][k] = (s_t[a][o], d_t[a][o])

    calls = []
    idx_cols = 0
    seg_cols = 0
    spans = {}            # (t, k) -> (idx_col0, seg_col0, slots, sizes)
    for t in range(cfg.NT):
        for k in range(SP):
            cnt = max(len(lists[c][t][k][0]) for c in range(cfg.NC))
            slots = -(-cnt // cfg.TP) * cfg.TP
            if slots == 0:
                continue
            # balanced chunks <= MAX_CALL, each a multiple of 128
            nch = -(-slots // MAX_CALL)
            base = slots // (nch * cfg.TP) * cfg.TP
            sizes = []
            rem = slots
            for i in range(nch):
                sz = rem if i == nch - 1 else min(rem, base + cfg.TP, MAX_CALL)
                sizes.append(sz)
                rem -= sz
            spans[(t, k)] = (idx_cols, seg_cols, slots, sizes)
            for sz in sizes:
                calls.append((t, k, sz, idx_cols, seg_cols))
                idx_cols += sz // 16
                seg_cols += sz // cfg.TP

    per_core = []
    for c in range(cfg.NC):
        idx = np.zeros((128, idx_cols), np.int16)
        seg = np.full((128, seg_cols), -1.0, np.float32)
        dnm = np.zeros((128, seg_cols), np.float32)
        for (t, k), (ic0, sc0, slots, sizes) in spans.items():
            s_flat, d_flat = lists[c][t][k]
            n = len(s_flat)
            buf_i = np.zeros(slots, np.int64)
            buf_s = -np.ones(slots, np.float32)
            buf_d = np.zeros(slots, np.float32)
            buf_i[:n] = s_flat - cfg.BOUNDS[k]
            buf_s[:n] = d_flat
            buf_d[:n] = dinv[s_flat]
            off = 0
            ic = ic0
            for sz in sizes:
                idx[:, ic:ic + sz // 16] = _wrap16(buf_i[off:off + sz])
                ic += sz // 16
                off += sz
            seg[:, sc0:sc0 + slots // cfg.TP] = (
                buf_s.reshape(slots // cfg.TP, cfg.TP).T)
            dnm[:, sc0:sc0 + slots // cfg.TP] = (
                buf_d.reshape(slots // cfg.TP, cfg.TP).T)
        per_core.append({"idx": idx.astype(np.int16),
                         "seg": seg.astype(BF16),
                         "dinvm": dnm.astype(BF16)})
    return deg, calls, idx_cols, seg_cols, per_core


def make_in_maps(inputs, cfg):
    """Build per-core input maps; returns (in_maps, calls, idx_cols, seg_cols)."""
    x = np.asarray(inputs["x"], np.float32)
    deg, calls, idx_cols, seg_cols, per_core = preprocess(
        np.asarray(inputs["edge_index"]), cfg)

    xp = np.zeros((cfg.NPAD, cfg.IN_CH), np.float32)
    xp[: x.shape[0]] = x
    xrows = xp.astype(BF16)                  # full gather table, replicated

    def chunks(a, k):  # [K*128, N] -> [K, 128, N]
        return np.ascontiguousarray(a.reshape(k, 128, a.shape[1]))

    wf = np.concatenate([np.asarray(inputs["sig_conv_w"], np.float32),
                         np.asarray(inputs["conv1_w"], np.float32)], axis=1)

    def aug(w, b):  # [N, K] weight + [N] bias -> [KA, 128, N] f32 (w.T | b | 0)
        wt = np.asarray(w, np.float32).T
        a = np.zeros((cfg.KA * 128, wt.shape[1]), np.float32)
        a[: wt.shape[0]] = wt
        a[wt.shape[0]] = np.asarray(b, np.float32)
        return chunks(a, cfg.KA)

    shared = {
        "xrows": xrows,
        "wf": chunks(wf, cfg.KX).astype(BF16),
        "w2": chunks(np.asarray(inputs["conv2_w"], np.float32),
                     cfg.KH).astype(BF16),
        "wg1": aug(inputs["fc1_w"], inputs["fc1_b"]),
        "wb1": aug(inputs["fc2_w"], inputs["fc2_b"]),
        "wg2": aug(inputs["fc3_w"], inputs["fc3_b"]),
        "wb2": aug(inputs["fc4_w"], inputs["fc4_b"]),
        "bsig": np.broadcast_to(np.asarray(inputs["sig_conv_b"], np.float32),
                                (128, cfg.HID)).copy(),
        "b1c": np.broadcast_to(np.asarray(inputs["conv1_b"], np.float32),
                               (128, cfg.HID)).copy(),
        "b2c": np.broadcast_to(np.asarray(inputs["conv2_b"], np.float32),
                               (128, cfg.OUT)).copy(),
        "iota": np.broadcast_to(np.arange(128, dtype=np.float32),
                                (128, 128)).astype(BF16).copy(),
        "ident": np.eye(128, dtype=np.float32).astype(BF16),
    }

    in_maps = []
    node_ids = np.arange(cfg.SHARD)
    for c in range(cfg.NC):
        sl = slice(c * cfg.SHARD, (c + 1) * cfg.SHARD)
        gids = node_ids + c * cfg.SHARD
        m = dict(shared)
        m["xloc"] = xrows[sl]                # this core's raw rows
        m["deg"] = _pmaj(deg[sl]).copy()
        m["sigmask"] = _pmaj((gids < cfg.n_real).astype(np.float32)).astype(BF16)
        m.update(per_core[c])
        in_maps.append(m)
    return in_maps, calls, idx_cols, seg_cols

# --------------------------------------------------------------- builder ----


def build_program(cfg, calls, idx_cols, seg_cols):
    nc = bacc.Bacc("TRN2", target_bir_lowering=False, debug=False,
                   num_devices=cfg.NC,
                   num_swdge_queues=NQ)
    f32, bf16, i16 = dt.float32, dt.bfloat16, dt.int16
    TP, NT = cfg.TP, cfg.NT
    HID, OUT, FUSED = cfg.HID, cfg.OUT, cfg.FUSED
    IN_CH = cfg.IN_CH
    MAXC = MAX_CALL // TP

    calls_by_tile = [[] for _ in range(NT)]
    for (t, k, sz, ic, sc) in calls:
        calls_by_tile[t].append((k, sz, ic, sc))
    NTA = (NT + 1) // 2
    RA = NTA * TP

    def inp(name, shape, dtype):
        return nc.dram_tensor(name, shape, dtype, kind="ExternalInput")

    xrows_d = inp("xrows", [cfg.NPAD, IN_CH], bf16)
    xloc_d = inp("xloc", [cfg.SHARD, IN_CH], bf16)
    wf_d = inp("wf", [cfg.KX, TP, FUSED], bf16)
    w2_d = inp("w2", [cfg.KH, TP, OUT], bf16)
    wg1_d = inp("wg1", [cfg.KA, TP, HID], f32)
    wb1_d = inp("wb1", [cfg.KA, TP, HID], f32)
    wg2_d = inp("wg2", [cfg.KA, TP, OUT], f32)
    wb2_d = inp("wb2", [cfg.KA, TP, OUT], f32)
    bsig_d = inp("bsig", [TP, HID], f32)
    b1c_d = inp("b1c", [TP, HID], f32)
    b2c_d = inp("b2c", [TP, OUT], f32)
    iota_d = inp("iota", [TP, TP], bf16)
    ident_d = inp("ident", [TP, TP], bf16)
    deg_d = inp("deg", [TP, NT], f32)
    mask_d = inp("sigmask", [TP, NT], bf16)
    idx_d = inp("idx", [TP, idx_cols], i16)
    seg_d = inp("seg", [TP, seg_cols], bf16)
    dinvm_d = inp("dinvm", [TP, seg_cols], bf16)

    out_d = nc.dram_tensor("out", [cfg.SHARD, OUT], f32, kind="ExternalOutput")

    # internal DRAM (collective bounce buffers)
    tsh_d = nc.dram_tensor("tsh", [cfg.SHARD, OUT], bf16)
    tfull_d = nc.dram_tensor("tfull", [cfg.NPAD, OUT], bf16,
                             addr_space="Shared")
    tfull_v = tfull_d.ap().rearrange("(c r) o -> c r o", c=cfg.NC)
    sin_d = nc.dram_tensor("sin", [1, HID], f32)
    sout_d = nc.dram_tensor("sout", [1, HID], f32, addr_space="Shared")

    rg = [list(range(cfg.NC))]

    with tile.TileContext(nc) as tc:
        with (
            tc.tile_pool(name="const", bufs=1) as const,
            tc.tile_pool(name="persist", bufs=1) as persist,
            tc.tile_pool(name="xload", bufs=3) as xload,
            tc.tile_pool(name="gat", bufs=6) as gat,
            tc.tile_pool(name="sbuild", bufs=6) as sbuild,
            tc.tile_pool(name="epi", bufs=3) as epi,
            tc.tile_pool(name="small", bufs=4) as small,
            tc.tile_pool(name="one", bufs=1) as one,
            tc.tile_pool(name="ps_agg", bufs=3, space="PSUM") as ps_agg,
            tc.tile_pool(name="ps_w", bufs=2, space="PSUM") as ps_w,
            tc.tile_pool(name="ps_sig", bufs=1, space="PSUM") as ps_sig,
            tc.tile_pool(name="ps_sm", bufs=2, space="PSUM") as ps_sm,
        ):
            # ---- load constants (gather-critical tables first) ------------
            iota_sb = const.tile([TP, TP], bf16)
            seg_sb = const.tile([TP, seg_cols], bf16)
            dinvm_sb = const.tile([TP, seg_cols], bf16)
            deg_sb = const.tile([TP, NT], f32)
            idx_sb = const.tile([TP, idx_cols], i16)
            # idx in 4 chunks so the first gathers start sooner
            icut = -(-idx_cols // 4)
            nc.sync.dma_start(out=idx_sb[:, 0:icut], in_=idx_d.ap()[:, 0:icut])
            for t_, d in ((iota_sb, iota_d), (seg_sb, seg_d),
                          (dinvm_sb, dinvm_d), (deg_sb, deg_d)):
                nc.sync.dma_start(out=t_[:], in_=d.ap())
            for i0 in range(icut, idx_cols, icut):
                i1 = min(idx_cols, i0 + icut)
                nc.sync.dma_start(out=idx_sb[:, i0:i1],
                                  in_=idx_d.ap()[:, i0:i1])
            wf_sb = const.tile([TP, cfg.KX, FUSED], bf16)
            w2_sb = const.tile([TP, cfg.KH, OUT], bf16)
            nc.sync.dma_start(out=wf_sb[:], in_=wf_d.ap().transpose([1, 0, 2]))
            nc.sync.dma_start(out=w2_sb[:], in_=w2_d.ap().transpose([1, 0, 2]))
            fc_sb = {}
            for nm, d, width in (("wg1", wg1_d, HID), ("wb1", wb1_d, HID),
                                 ("wg2", wg2_d, OUT), ("wb2", wb2_d, OUT)):
                t_ = const.tile([TP, cfg.KA, width], f32, name=nm)
                nc.sync.dma_start(out=t_[:], in_=d.ap().transpose([1, 0, 2]))
                fc_sb[nm] = t_
            bsig_sb = const.tile([TP, HID], f32)
            b1c_sb = const.tile([TP, HID], f32)
            b2c_sb = const.tile([TP, OUT], f32)
            ident_sb = const.tile([TP, TP], bf16)
            mask_sb = const.tile([TP, NT], bf16)
            for t_, d in ((bsig_sb, bsig_d), (b1c_sb, b1c_d), (b2c_sb, b2c_d),
                          (ident_sb, ident_d), (mask_sb, mask_d)):
                nc.sync.dma_start(out=t_[:], in_=d.ap())

            eps_sb = const.tile([TP, 1], f32)
            nc.vector.memset(eps_sb[:], cfg.LN_EPS)

            # dinv = 1/sqrt(deg)
            dinv_sb = const.tile([TP, NT], f32)
            nc.scalar.sqrt(dinv_sb[:], deg_sb[:])
            nc.vector.reciprocal(dinv_sb[:], dinv_sb[:])

            # persistent per-shard state
            c1agg_sb = persist.tile([TP, NT, HID], bf16)    # conv1 of z
            tp_sb = persist.tile([TP, NT, OUT], bf16)       # t' shard

            s_ps = ps_sig.tile([1, HID], f32)
            qctr = [0]

            def seg_gather(t, table_d, width, ps_pool, tag, fold_dinv):
                """gather + segsum-matmul for dst-tile t; returns psum tile
                (or None when the tile has no in-edges)."""
                tcalls = calls_by_tile[t]
                if not tcalls:
                    return None
                ps = ps_pool.tile([TP, width], f32, tag=tag)
                nmm = sum(sz // TP for (_, sz, _, _) in tcalls)
                mm = 0
                for (k, sz, ic, sc) in tcalls:
                    table = table_d.ap()[cfg.BOUNDS[k]:cfg.BOUNDS[k + 1], :]
                    cch = sz // TP
                    g = gat.tile([TP, MAXC, width], bf16, tag=f"g{width}",
                                 name=f"g_{t}_{k}")
                    nc.gpsimd.dma_gather(
                        out_ap=g[:, :cch, :],
                        in_ap=table,
                        idxs_ap=idx_sb[:, ic:ic + sz // 16],
                        num_idxs=sz,
                        num_idxs_reg=sz,
                        elem_size=width,
                        queue_num=qctr[0] % NQ,
                    )
                    qctr[0] += 1
                    S = sbuild.tile([TP, MAXC, TP], bf16, tag="S",
                                    name=f"S_{t}_{k}")
                    seg_col = seg_sb[:, sc:sc + cch]
                    nc.vector.tensor_tensor(
                        S[:, :cch, :],
                        seg_col.unsqueeze(2).to_broadcast((TP, cch, TP)),
                        iota_sb[:].unsqueeze(1).to_broadcast((TP, cch, TP)),
                        mybir.AluOpType.is_equal)
                    if fold_dinv:
                        dm_col = dinvm_sb[:, sc:sc + cch]
                        nc.vector.tensor_tensor(
                            S[:, :cch, :],
                            S[:, :cch, :],
                            dm_col.unsqueeze(2).to_broadcast((TP, cch, TP)),
                            mybir.AluOpType.mult)
                    for j in range(cch):
                        nc.tensor.matmul(ps[:], S[:, j, :], g[:, j, :],
                                         start=(mm == 0),
                                         stop=(mm == nmm - 1))
                        mm += 1
                return ps

            # ---- pass 1: aggregate raw x, then z @ [Wsig|W1] --------------
            with nc.named_scope("pass1"):
                for t in range(NT):
                    ps = seg_gather(t, xrows_d, IN_CH, ps_agg, "agg", True)
                    dv = dinv_sb[:, t:t + 1]
                    xl = xload.tile([TP, IN_CH], bf16, tag="xl",
                                    name=f"xl_{t}")
                    nc.sync.dma_start(
                        out=xl[:], in_=xloc_d.ap()[t * TP:(t + 1) * TP, :])
                    # z = dv * (agg + dv * x_self)
                    zt = epi.tile([TP, IN_CH], f32, tag="zt", name=f"zt_{t}")
                    if ps is not None:
                        nc.vector.scalar_tensor_tensor(
                            zt[:], xl[:], dv, ps[:],
                            mybir.AluOpType.mult, mybir.AluOpType.add)
                    else:
                        nc.vector.scalar_tensor_tensor(
                            zt[:], xl[:], dv, xl[:],
                            mybir.AluOpType.mult, mybir.AluOpType.bypass)
                    zb = epi.tile([TP, IN_CH], bf16, tag="zb", name=f"zb_{t}")
                    nc.scalar.activation(zb[:], zt[:],
                                         mybir.ActivationFunctionType.Copy,
                                         scale=dv)
                    # transpose z and apply the fused weight
                    zT = epi.tile([TP, cfg.KX, TP], bf16, tag="zT",
                                  name=f"zT_{t}")
                    for c in range(cfg.KX):
                        ps_t = ps_sm.tile([TP, TP], bf16, tag="sm",
                                          name=f"ztr_{t}_{c}")
                        nc.tensor.transpose(ps_t[:],
                                            zb[:, c * TP:(c + 1) * TP],
                                            ident_sb[:])
                        nc.vector.tensor_copy(zT[:, c, :], ps_t[:])
                    ps2 = ps_w.tile([TP, FUSED], f32, tag="w")
                    for c in range(cfg.KX):
                        nc.tensor.matmul(ps2[:], zT[:, c, :], wf_sb[:, c, :],
                                         start=(c == 0), stop=(c == cfg.KX - 1))
                    # sig half: relu(. + bsig); mask-sum on PE
                    sig_f = epi.tile([TP, HID], f32, tag="sigf")
                    nc.vector.tensor_tensor(sig_f[:], ps2[:, :HID],
                                            bsig_sb[:], mybir.AluOpType.add)
                    sig_b = epi.tile([TP, HID], bf16, tag="sigb")
                    nc.scalar.activation(sig_b[:], sig_f[:],
                                         mybir.ActivationFunctionType.Relu)
                    nc.tensor.matmul(s_ps[:], mask_sb[:, t:t + 1], sig_b[:],
                                     start=(t == 0), stop=(t == NT - 1))
                    # conv1 half
                    nc.scalar.copy(c1agg_sb[:, t, :], ps2[:, HID:])

            # ---- signature allreduce + gamma/beta -------------------------
            with nc.named_scope("signature"):
                s_sb = one.tile([1, HID], f32)
                nc.scalar.copy(s_sb[:], s_ps[:])
                nc.sync.dma_start(out=sin_d.ap(), in_=s_sb[:])
                nc.gpsimd.collective_compute(
                    "AllReduce", mybir.AluOpType.add, replica_groups=rg,
                    ins=[sin_d.ap().opt()], outs=[sout_d.ap().opt()])

                s_col = one.tile([TP, cfg.KA], f32)
                nc.vector.memset(s_col[:], 0.0)
                nc.vector.memset(s_col[0:1, cfg.KA - 1:cfg.KA], 1.0)
                nc.sync.dma_start(
                    out=s_col[:, 0:2],
                    in_=sout_d.ap().rearrange("o (c p) -> (o c) p", p=TP)
                        .transpose([1, 0]))
                s_rep = one.tile([TP, cfg.KA, TP], f32)
                for c in range(cfg.KA):
                    nc.vector.tensor_copy(
                        s_rep[:, c, :],
                        s_col[:, c:c + 1].to_broadcast((TP, TP)))

                gb_sb = {}
                for nm, width in (("wg1", HID), ("wb1", HID),
                                  ("wg2", OUT), ("wb2", OUT)):
                    ps_fc = ps_sm.tile([TP, width], f32, tag="sm", name=nm)
                    for c in range(cfg.KA):
                        nc.tensor.matmul(ps_fc[:], s_rep[:, c, :],
                                         fc_sb[nm][:, c, :],
                                         start=(c == 0), stop=(c == cfg.KA - 1))
                    gb = one.tile([TP, width], f32, name=f"gb_{nm}", tag=nm)
                    nc.scalar.activation(gb[:], ps_fc[:],
                                         mybir.ActivationFunctionType.Tanh)
                    gb_sb[nm] = gb
                # beta + conv bias
                nc.vector.tensor_tensor(gb_sb["wb1"][:], gb_sb["wb1"][:],
                                        b1c_sb[:], mybir.AluOpType.add)
                nc.vector.tensor_tensor(gb_sb["wb2"][:], gb_sb["wb2"][:],
                                        b2c_sb[:], mybir.AluOpType.add)

            # ---- encoder local: FiLM + relu + LN + conv2 matmul -----------
            def layernorm(dst_ap, src_ap, width):
                st6 = small.tile([TP, 6], f32, tag="st6", name="st6")
                mv = small.tile([TP, 2], f32, tag="mv", name="mv")
                nc.vector.bn_stats(st6[:], src_ap)
                nc.vector.bn_aggr(mv[:], st6[:])
                std = small.tile([TP, 1], f32, tag="std", name="std")
                nc.scalar.activation(std[:], mv[:, 1:2],
                                     mybir.ActivationFunctionType.Sqrt,
                                     bias=eps_sb[:, 0:1])
                rstd = small.tile([TP, 1], f32, tag="rstd", name="rstd")
                nc.vector.reciprocal(rstd[:], std[:])
                nmr = small.tile([TP, 1], f32, tag="nmr", name="nmr")
                nc.vector.scalar_tensor_tensor(
                    nmr[:], mv[:, 0:1], -1.0, rstd[:],
                    mybir.AluOpType.mult, mybir.AluOpType.mult)
                nc.scalar.activation(dst_ap, src_ap,
                                     mybir.ActivationFunctionType.Identity,
                                     bias=nmr[:, 0:1], scale=rstd[:, 0:1])

            with nc.named_scope("encoder_local"):
                # batched FiLM + relu + LN stats in groups (fewer, larger
                # DVE/Act ops; only one Act per tile stays on the chain)
                GRP = 8
                hr_sb = persist.tile([TP, NT, HID], bf16)
                sums = one.tile([TP, NT], f32)
                sumsq = one.tile([TP, NT], f32)
                for g0 in range(0, NT, GRP):
                    gn = min(GRP, NT - g0)
                    hf_g = epi.tile([TP, GRP, HID], f32, tag="hfg",
                                    name=f"hf_{g0}")
                    nc.vector.tensor_tensor(
                        hf_g[:, :gn, :], c1agg_sb[:, g0:g0 + gn, :],
                        gb_sb["wg1"][:].unsqueeze(1)
                        .to_broadcast((TP, gn, HID)),
                        mybir.AluOpType.mult)
                    nc.vector.tensor_tensor(
                        hf_g[:, :gn, :], hf_g[:, :gn, :],
                        gb_sb["wb1"][:].unsqueeze(1)
                        .to_broadcast((TP, gn, HID)),
                        mybir.AluOpType.add)
                    nc.scalar.activation(hr_sb[:, g0:g0 + gn, :],
                                         hf_g[:, :gn, :],
                                         mybir.ActivationFunctionType.Relu)
                    sq_g = epi.tile([TP, GRP, HID], bf16, tag="sqg",
                                    name=f"sq_{g0}")
                    nc.vector.tensor_tensor(
                        sq_g[:, :gn, :], hr_sb[:, g0:g0 + gn, :],
                        hr_sb[:, g0:g0 + gn, :], mybir.AluOpType.mult)
                    nc.vector.tensor_reduce(
                        sums[:, g0:g0 + gn], hr_sb[:, g0:g0 + gn, :],
                        mybir.AxisListType.X, mybir.AluOpType.add)
                    nc.vector.tensor_reduce(
                        sumsq[:, g0:g0 + gn], sq_g[:, :gn, :],
                        mybir.AxisListType.X, mybir.AluOpType.add)
                # mean/rstd for all tiles at once
                mean_a = one.tile([TP, NT], f32)
                var_a = one.tile([TP, NT], f32)
                rstd_a = one.tile([TP, NT], f32)
                nmr_a = one.tile([TP, NT], f32)
                nc.scalar.activation(mean_a[:], sums[:],
                                     mybir.ActivationFunctionType.Copy,
                                     scale=1.0 / HID)
                nc.scalar.activation(var_a[:], sumsq[:],
                                     mybir.ActivationFunctionType.Copy,
                                     scale=1.0 / HID)
                nc.vector.tensor_tensor(nmr_a[:], mean_a[:], mean_a[:],
                                        mybir.AluOpType.mult)
                nc.vector.tensor_tensor(var_a[:], var_a[:], nmr_a[:],
                                        mybir.AluOpType.subtract)
                nc.scalar.activation(var_a[:], var_a[:],
                                     mybir.ActivationFunctionType.Sqrt,
                                     bias=eps_sb[:, 0:1])
                nc.vector.reciprocal(rstd_a[:], var_a[:])
                nc.vector.scalar_tensor_tensor(
                    nmr_a[:], mean_a[:], -1.0, rstd_a[:],
                    mybir.AluOpType.mult, mybir.AluOpType.mult)
                for t in range(NT):
                    h1 = epi.tile([TP, HID], bf16, tag="h1", name=f"h1_{t}")
                    nc.scalar.activation(h1[:], hr_sb[:, t, :],
                                         mybir.ActivationFunctionType.Identity,
                                         bias=nmr_a[:, t:t + 1],
                                         scale=rstd_a[:, t:t + 1])
                    # transpose h1 tile and matmul with w2
                    h1T = epi.tile([TP, cfg.KH, TP], bf16, tag="h1T",
                                   name=f"h1T_{t}")
                    for c in range(cfg.KH):
                        ps_t = ps_sm.tile([TP, TP], bf16, tag="sm",
                                          name=f"tr_{t}_{c}")
                        nc.tensor.transpose(ps_t[:],
                                            h1[:, c * TP:(c + 1) * TP],
                                            ident_sb[:])
                        nc.vector.tensor_copy(h1T[:, c, :], ps_t[:])
                    ps2 = ps_w.tile([TP, OUT], f32, tag="w", name=f"w2_{t}")
                    for c in range(cfg.KH):
                        nc.tensor.matmul(ps2[:], h1T[:, c, :], w2_sb[:, c, :],
                                         start=(c == 0), stop=(c == cfg.KH - 1))
                    nc.scalar.activation(tp_sb[:, t, :], ps2[:],
                                         mybir.ActivationFunctionType.Copy,
                                         scale=dinv_sb[:, t:t + 1])
                    nc.sync.dma_start(out=tsh_d.ap()[t * TP:(t + 1) * TP, :],
                                      in_=tp_sb[:, t, :])
                nc.gpsimd.collective_compute(
                    "AllGather", mybir.AluOpType.bypass, replica_groups=rg,
                    ins=[tsh_d.ap().opt()], outs=[tfull_d.ap().opt()])

            # ---- pass 2: edge aggregation over t' -------------------------
            with nc.named_scope("pass2"):
                for t in range(NT):
                    ps = seg_gather(t, tfull_d, OUT, ps_agg, "agg", False)
                    dv = dinv_sb[:, t:t + 1]
                    o_f = epi.tile([TP, OUT], f32, tag="of", name=f"o_{t}")
                    if ps is not None:
                        nc.vector.tensor_tensor(o_f[:], ps[:],
                                                tp_sb[:, t, :],
                                                mybir.AluOpType.add)
                    else:
                        nc.vector.tensor_copy(o_f[:], tp_sb[:, t, :])
                    # gamma2 * (dinv * agg) + (beta2 + b2)
                    nc.vector.scalar_tensor_tensor(
                        o_f[:], o_f[:], dv, gb_sb["wg2"][:],
                        mybir.AluOpType.mult, mybir.AluOpType.mult)
                    nc.vector.tensor_tensor(o_f[:], o_f[:], gb_sb["wb2"][:],
                                            mybir.AluOpType.add)
                    o_ln = epi.tile([TP, OUT], f32, tag="oln", name=f"ol_{t}")
                    layernorm(o_ln[:], o_f[:], OUT)
                    nc.sync.dma_start(out=out_d.ap()[t * TP:(t + 1) * TP, :],
                                      in_=o_ln[:])

    nc.compile()
    return nc

# ---------------------------------------------------------------- runner ----


_CACHE = {}


def _get_program(cfg, calls, idx_cols, seg_cols):
    key = (cfg.NT, tuple(calls))
    if key not in _CACHE:
        _CACHE[key] = build_program(cfg, calls, idx_cols, seg_cols)
    return _CACHE[key]


def run(inputs, cfg=FULL, trace=False, **kw):
    in_maps, calls, idx_cols, seg_cols = make_in_maps(inputs, cfg)
    nc = _get_program(cfg, calls, idx_cols, seg_cols)
    res = bass_utils.run_bass_kernel_spmd(
        nc, in_maps, core_ids=list(range(cfg.NC)), trace=trace, **kw)
    out = np.concatenate([res.results[c]["out"] for c in range(cfg.NC)],
                         axis=0)[: cfg.n_real]
    return out.astype(np.float32), res


def kernel(**inputs):
    out, _ = run(inputs, FULL)
    return out


# revision 30
# speedup vs baseline: 1.0310x; 1.0310x over previous
"""Trainium2 Bass kernel for nn_MetaSignatureEncoder (GCN encoder with FiLM
signature conditioning), distributed over 8 NeuronCores.

Strategy (graph/data parallel, per the sharding hint):
  - Nodes are padded to NPAD = 8*49*128 = 50176 and sharded contiguously
    across the 8 cores (6272 nodes/core, 49 dst tiles of 128).
  - GCN linearity is exploited twice: norm[e] = dinv[src]*dinv[dst] factors
    out of the message sum, and (A x) @ W == A (x @ W), so pass 1 aggregates
    RAW x rows (256ch bf16, staged replicated to every core's HBM - no
    device-side table build, no collective) and applies [Wsig|W1] to the
    128-node aggregate tile afterwards.  dinv[src] rides in the one-hot
    scatter matrix; dinv[dst] is applied to the aggregate.
  - Pass 1 (edges): per dst-tile of 128 nodes, dma_gather fetches the x rows
    of all in-edges.  Gather calls are sized to the REAL per-(tile,segment)
    edge counts (max across cores, rounded to 128, 3 src-range segments for
    int16 indices, chunked <=1024 - the HW limit).  4 SWDGE queues
    round-robin.  The segment-sum runs on the TensorEngine: for each chunk
    of 128 messages an S matrix (S[j,d] = dinv[src_j] if dst_j == d, built
    by DVE is_equal + multiply) matmuls the gathered rows into PSUM.
  - The graph signature s = sum_n relu(z @ Wsig + b) is reduced over nodes
    with a mask-vector matmul (PE) and AllReduce'd in f32.
  - gamma/beta FiLM vectors are computed redundantly on every core.
  - Encoder: FiLM + relu + LN per tile, conv2 matmul via PE transpose,
    dinv scale, AllGather of the [NPAD, 128] bf16 t' table, second edge
    pass (structure as pass 1 but with pre-scaled 128ch rows and plain
    one-hot S), FiLM + LN epilogue, output.

kernel(**inputs) takes the FULL problem inputs and returns the FULL output.
"""
import sys
import numpy as np
import ml_dtypes

sys.path.insert(0, "/opt/trn_rl_repo")

from concourse import bass, bacc, tile, mybir
from concourse import bass_utils

BF16 = ml_dtypes.bfloat16
dt = mybir.dt

MAX_CALL = 1024          # HW limit on num_idxs per dma_gather
NQ = 4                   # SWDGE queues

# ---------------------------------------------------------------- config ----


class Cfg:
    def __init__(self, NT=49, n_real=50000):
        self.NC = 8           # cores
        self.TP = 128         # partitions / dst-tile size
        self.NT = NT          # dst tiles per core
        self.SPLITS = 3       # src-space segments (int16 idx range)
        self.IN_CH = 256
        self.HID = 256
        self.OUT = 128
        self.FUSED = self.HID + self.HID   # sig(256) | conv1(256)
        self.KX = self.IN_CH // 128        # K chunks for x matmuls
        self.KH = self.HID // 128          # K chunks for conv2 matmul
        self.KA = 3                        # K chunks for augmented fc matmuls
        self.SHARD = self.NT * self.TP
        self.NPAD = self.NC * self.SHARD
        self.BOUNDS = [self.NPAD * k // self.SPLITS
                       for k in range(self.SPLITS)] + [self.NPAD]
        self.n_real = n_real
        self.LN_EPS = 1e-5


FULL = Cfg()

# ------------------------------------------------------------ host side -----


def _wrap16(vals, nrows=128):
    """dma_gather index layout: idx j at [j % 16, j // 16], replicated to all
    8 q7 core groups (rows 16k+p == row p)."""
    n = vals.shape[0]
    assert n % 16 == 0
    w = vals.reshape(n // 16, 16).T          # [16, n/16]
    return np.tile(w, (nrows // 16, 1))      # [128, n/16]


def _pmaj(vals, TP=128):
    """[NT*TP] -> [TP, NT] partition-major (tile t col, partition p row)."""
    return np.ascontiguousarray(vals.reshape(-1, TP).T)


def preprocess(edge_index, cfg):
    """Integer-only graph preprocessing -> call plan + per-core indices.

    Returns (deg, calls, idx_cols, seg_cols, per_core); calls is the SHARED
    plan [(t, k, slots, idx_off, seg_off)] with slots = max real count over
    cores rounded to 128, balanced-chunked <= MAX_CALL.
    """
    src = np.asarray(edge_index[0], dtype=np.int64)
    dst = np.asarray(edge_index[1], dtype=np.int64)
    deg = np.bincount(src, minlength=cfg.NPAD).astype(np.float32) + 1.0
    dinv = deg ** -0.5

    SP = cfg.SPLITS
    lists = [[[None] * SP for _ in range(cfg.NT)] for _ in range(cfg.NC)]
    shard_of = dst // cfg.SHARD
    for c in range(cfg.NC):
        m = shard_of == c
        s_c = src[m]
        d_c = dst[m] - c * cfg.SHARD
        tile_of = d_c // cfg.TP
        order = np.argsort(tile_of, kind="stable")
        s_c, d_c, tile_of = s_c[order], d_c[order], tile_of[order]
        bounds = np.searchsorted(tile_of, np.arange(cfg.NT + 1))
        for t in range(cfg.NT):
            sl = slice(bounds[t], bounds[t + 1])
            s_t = s_c[sl]
            d_t = d_c[sl] - t * cfg.TP
            for k in range(SP):
                a = (s_t >= cfg.BOUNDS[k]) & (s_t < cfg.BOUNDS[k + 1])
                o = np.argsort(s_t[a], kind="stable")  # src order: locality
                lists[c][t][k] = (s_t[a][o], d_t[a][o])

    calls = []
    idx_cols = 0
    seg_cols = 0
    spans = {}            # (t, k) -> (idx_col0, seg_col0, slots, sizes)
    for t in range(cfg.NT):
        for k in range(SP):
            cnt = max(len(lists[c][t][k][0]) for c in range(cfg.NC))
            slots = -(-cnt // cfg.TP) * cfg.TP
            if slots == 0:
                continue
            # balanced chunks <= MAX_CALL, each a multiple of 128
            nch = -(-slots // MAX_CALL)
            base = slots // (nch * cfg.TP) * cfg.TP
            sizes = []
            rem = slots
            for i in range(nch):
                sz = rem if i == nch - 1 else min(rem, base + cfg.TP, MAX_CALL)
                sizes.append(sz)
                rem -= sz
            spans[(t, k)] = (idx_cols, seg_cols, slots, sizes)
            for sz in sizes:
                calls.append((t, k, sz, idx_cols, seg_cols))
                idx_cols += sz // 16
                seg_cols += sz // cfg.TP

    per_core = []
    for c in range(cfg.NC):
        idx = np.zeros((128, idx_cols), np.int16)
        seg = np.full((128, seg_cols), -1.0, np.float32)
        dnm = np.zeros((128, seg_cols), np.float32)
        for (t, k), (ic0, sc0, slots, sizes) in spans.items():
            s_flat, d_flat = lists[c][t][k]
            n = len(s_flat)
            buf_i = np.zeros(slots, np.int64)
            buf_s = -np.ones(slots, np.float32)
            buf_d = np.zeros(slots, np.float32)
            buf_i[:n] = s_flat - cfg.BOUNDS[k]
            buf_s[:n] = d_flat
            buf_d[:n] = dinv[s_flat]
            off = 0
            ic = ic0
            for sz in sizes:
                idx[:, ic:ic + sz // 16] = _wrap16(buf_i[off:off + sz])
                ic += sz // 16
                off += sz
            seg[:, sc0:sc0 + slots // cfg.TP] = (
                buf_s.reshape(slots // cfg.TP, cfg.TP).T)
            dnm[:, sc0:sc0 + slots // cfg.TP] = (
                buf_d.reshape(slots // cfg.TP, cfg.TP).T)
        per_core.append({"idx": idx.astype(np.int16),
                         "seg": seg.astype(BF16),
                         "dinvm": dnm.astype(BF16)})
    return deg, calls, idx_cols, seg_cols, per_core


def make_in_maps(inputs, cfg):
    """Build per-core input maps; returns (in_maps, calls, idx_cols, seg_cols)."""
    x = np.asarray(inputs["x"], np.float32)
    deg, calls, idx_cols, seg_cols, per_core = preprocess(
        np.asarray(inputs["edge_index"]), cfg)

    xp = np.zeros((cfg.NPAD, cfg.IN_CH), np.float32)
    xp[: x.shape[0]] = x
    xrows = xp.astype(BF16)                  # full gather table, replicated

    def chunks(a, k):  # [K*128, N] -> [K, 128, N]
        return np.ascontiguousarray(a.reshape(k, 128, a.shape[1]))

    wf = np.concatenate([np.asarray(inputs["sig_conv_w"], np.float32),
                         np.asarray(inputs["conv1_w"], np.float32)], axis=1)

    def aug(w, b):  # [N, K] weight + [N] bias -> [KA, 128, N] f32 (w.T | b | 0)
        wt = np.asarray(w, np.float32).T
        a = np.zeros((cfg.KA * 128, wt.shape[1]), np.float32)
        a[: wt.shape[0]] = wt
        a[wt.shape[0]] = np.asarray(b, np.float32)
        return chunks(a, cfg.KA)

    shared = {
        "xrows": xrows,
        "wf": chunks(wf, cfg.KX).astype(BF16),
        "w2": chunks(np.asarray(inputs["conv2_w"], np.float32),
                     cfg.KH).astype(BF16),
        "wg1": aug(inputs["fc1_w"], inputs["fc1_b"]),
        "wb1": aug(inputs["fc2_w"], inputs["fc2_b"]),
        "wg2": aug(inputs["fc3_w"], inputs["fc3_b"]),
        "wb2": aug(inputs["fc4_w"], inputs["fc4_b"]),
        "bsig": np.broadcast_to(np.asarray(inputs["sig_conv_b"], np.float32),
                                (128, cfg.HID)).copy(),
        "b1c": np.broadcast_to(np.asarray(inputs["conv1_b"], np.float32),
                               (128, cfg.HID)).copy(),
        "b2c": np.broadcast_to(np.asarray(inputs["conv2_b"], np.float32),
                               (128, cfg.OUT)).copy(),
        "iota": np.broadcast_to(np.arange(128, dtype=np.float32),
                                (128, 128)).astype(BF16).copy(),
        "ident": np.eye(128, dtype=np.float32).astype(BF16),
    }

    in_maps = []
    node_ids = np.arange(cfg.SHARD)
    for c in range(cfg.NC):
        sl = slice(c * cfg.SHARD, (c + 1) * cfg.SHARD)
        gids = node_ids + c * cfg.SHARD
        m = dict(shared)
        m["xloc"] = xrows[sl]                # this core's raw rows
        m["deg"] = _pmaj(deg[sl]).copy()
        m["sigmask"] = _pmaj((gids < cfg.n_real).astype(np.float32)).astype(BF16)
        m.update(per_core[c])
        in_maps.append(m)
    return in_maps, calls, idx_cols, seg_cols

# --------------------------------------------------------------- builder ----


def build_program(cfg, calls, idx_cols, seg_cols):
    nc = bacc.Bacc("TRN2", target_bir_lowering=False, debug=False,
                   num_devices=cfg.NC,
                   num_swdge_queues=NQ)
    f32, bf16, i16 = dt.float32, dt.bfloat16, dt.int16
    TP, NT = cfg.TP, cfg.NT
    HID, OUT, FUSED = cfg.HID, cfg.OUT, cfg.FUSED
    IN_CH = cfg.IN_CH
    MAXC = MAX_CALL // TP

    calls_by_tile = [[] for _ in range(NT)]
    for (t, k, sz, ic, sc) in calls:
        calls_by_tile[t].append((k, sz, ic, sc))
    NTA = (NT + 1) // 2
    RA = NTA * TP

    def inp(name, shape, dtype):
        return nc.dram_tensor(name, shape, dtype, kind="ExternalInput")

    xrows_d = inp("xrows", [cfg.NPAD, IN_CH], bf16)
    xloc_d = inp("xloc", [cfg.SHARD, IN_CH], bf16)
    wf_d = inp("wf", [cfg.KX, TP, FUSED], bf16)
    w2_d = inp("w2", [cfg.KH, TP, OUT], bf16)
    wg1_d = inp("wg1", [cfg.KA, TP, HID], f32)
    wb1_d = inp("wb1", [cfg.KA, TP, HID], f32)
    wg2_d = inp("wg2", [cfg.KA, TP, OUT], f32)
    wb2_d = inp("wb2", [cfg.KA, TP, OUT], f32)
    bsig_d = inp("bsig", [TP, HID], f32)
    b1c_d = inp("b1c", [TP, HID], f32)
    b2c_d = inp("b2c", [TP, OUT], f32)
    iota_d = inp("iota", [TP, TP], bf16)
    ident_d = inp("ident", [TP, TP], bf16)
    deg_d = inp("deg", [TP, NT], f32)
    mask_d = inp("sigmask", [TP, NT], bf16)
    idx_d = inp("idx", [TP, idx_cols], i16)
    seg_d = inp("seg", [TP, seg_cols], bf16)
    dinvm_d = inp("dinvm", [TP, seg_cols], bf16)

    out_d = nc.dram_tensor("out", [cfg.SHARD, OUT], f32, kind="ExternalOutput")

    # internal DRAM (collective bounce buffers)
    tsh_d = nc.dram_tensor("tsh", [cfg.SHARD, OUT], bf16)
    tfull_d = nc.dram_tensor("tfull", [cfg.NPAD, OUT], bf16,
                             addr_space="Shared")
    tfull_v = tfull_d.ap().rearrange("(c r) o -> c r o", c=cfg.NC)
    sin_d = nc.dram_tensor("sin", [1, HID], f32)
    sout_d = nc.dram_tensor("sout", [1, HID], f32, addr_space="Shared")

    rg = [list(range(cfg.NC))]

    with tile.TileContext(nc) as tc:
        with (
            tc.tile_pool(name="const", bufs=1) as const,
            tc.tile_pool(name="persist", bufs=1) as persist,
            tc.tile_pool(name="xload", bufs=3) as xload,
            tc.tile_pool(name="gat", bufs=6) as gat,
            tc.tile_pool(name="sbuild", bufs=6) as sbuild,
            tc.tile_pool(name="epi", bufs=3) as epi,
            tc.tile_pool(name="small", bufs=4) as small,
            tc.tile_pool(name="one", bufs=1) as one,
            tc.tile_pool(name="ps_agg", bufs=3, space="PSUM") as ps_agg,
            tc.tile_pool(name="ps_w", bufs=2, space="PSUM") as ps_w,
            tc.tile_pool(name="ps_sig", bufs=1, space="PSUM") as ps_sig,
            tc.tile_pool(name="ps_sm", bufs=2, space="PSUM") as ps_sm,
        ):
            # ---- load constants (gather-critical tables first) ------------
            iota_sb = const.tile([TP, TP], bf16)
            seg_sb = const.tile([TP, seg_cols], bf16)
            dinvm_sb = const.tile([TP, seg_cols], bf16)
            deg_sb = const.tile([TP, NT], f32)
            idx_sb = const.tile([TP, idx_cols], i16)
            # idx in 4 chunks so the first gathers start sooner
            icut = -(-idx_cols // 4)
            nc.sync.dma_start(out=idx_sb[:, 0:icut], in_=idx_d.ap()[:, 0:icut])
            for t_, d in ((iota_sb, iota_d), (seg_sb, seg_d),
                          (dinvm_sb, dinvm_d), (deg_sb, deg_d)):
                nc.sync.dma_start(out=t_[:], in_=d.ap())
            for i0 in range(icut, idx_cols, icut):
                i1 = min(idx_cols, i0 + icut)
                nc.sync.dma_start(out=idx_sb[:, i0:i1],
                                  in_=idx_d.ap()[:, i0:i1])
            wf_sb = const.tile([TP, cfg.KX, FUSED], bf16)
            w2_sb = const.tile([TP, cfg.KH, OUT], bf16)
            nc.sync.dma_start(out=wf_sb[:], in_=wf_d.ap().transpose([1, 0, 2]))
            nc.sync.dma_start(out=w2_sb[:], in_=w2_d.ap().transpose([1, 0, 2]))
            fc_sb = {}
            for nm, d, width in (("wg1", wg1_d, HID), ("wb1", wb1_d, HID),
                                 ("wg2", wg2_d, OUT), ("wb2", wb2_d, OUT)):
                t_ = const.tile([TP, cfg.KA, width], f32, name=nm)
                nc.sync.dma_start(out=t_[:], in_=d.ap().transpose([1, 0, 2]))
                fc_sb[nm] = t_
            bsig_sb = const.tile([TP, HID], f32)
            b1c_sb = const.tile([TP, HID], f32)
            b2c_sb = const.tile([TP, OUT], f32)
            ident_sb = const.tile([TP, TP], bf16)
            mask_sb = const.tile([TP, NT], bf16)
            for t_, d in ((bsig_sb, bsig_d), (b1c_sb, b1c_d), (b2c_sb, b2c_d),
                          (ident_sb, ident_d), (mask_sb, mask_d)):
                nc.sync.dma_start(out=t_[:], in_=d.ap())

            eps_sb = const.tile([TP, 1], f32)
            nc.vector.memset(eps_sb[:], cfg.LN_EPS)

            # dinv = 1/sqrt(deg)
            dinv_sb = const.tile([TP, NT], f32)
            nc.scalar.sqrt(dinv_sb[:], deg_sb[:])
            nc.vector.reciprocal(dinv_sb[:], dinv_sb[:])

            # persistent per-shard state
            c1agg_sb = persist.tile([TP, NT, HID], bf16)    # conv1 of z
            tp_sb = persist.tile([TP, NT, OUT], bf16)       # t' shard

            s_ps = ps_sig.tile([1, HID], f32)
            qctr = [0]

            def seg_gather(t, table_d, width, ps_pool, tag, fold_dinv):
                """gather + segsum-matmul for dst-tile t; returns psum tile
                (or None when the tile has no in-edges)."""
                tcalls = calls_by_tile[t]
                if not tcalls:
                    return None
                ps = ps_pool.tile([TP, width], f32, tag=tag)
                nmm = sum(sz // TP for (_, sz, _, _) in tcalls)
                mm = 0
                for (k, sz, ic, sc) in tcalls:
                    table = table_d.ap()[cfg.BOUNDS[k]:cfg.BOUNDS[k + 1], :]
                    cch = sz // TP
                    g = gat.tile([TP, MAXC, width], bf16, tag=f"g{width}",
                                 name=f"g_{t}_{k}")
                    nc.gpsimd.dma_gather(
                        out_ap=g[:, :cch, :],
                        in_ap=table,
                        idxs_ap=idx_sb[:, ic:ic + sz // 16],
                        num_idxs=sz,
                        num_idxs_reg=sz,
                        elem_size=width,
                        queue_num=qctr[0] % NQ,
                    )
                    qctr[0] += 1
                    S = sbuild.tile([TP, MAXC, TP], bf16, tag="S",
                                    name=f"S_{t}_{k}")
                    seg_col = seg_sb[:, sc:sc + cch]
                    nc.vector.tensor_tensor(
                        S[:, :cch, :],
                        seg_col.unsqueeze(2).to_broadcast((TP, cch, TP)),
                        iota_sb[:].unsqueeze(1).to_broadcast((TP, cch, TP)),
                        mybir.AluOpType.is_equal)
                    if fold_dinv:
                        dm_col = dinvm_sb[:, sc:sc + cch]
                        nc.vector.tensor_tensor(
                            S[:, :cch, :],
                            S[:, :cch, :],
                            dm_col.unsqueeze(2).to_broadcast((TP, cch, TP)),
                            mybir.AluOpType.mult)
                    for j in range(cch):
                        nc.tensor.matmul(ps[:], S[:, j, :], g[:, j, :],
                                         start=(mm == 0),
                                         stop=(mm == nmm - 1))
                        mm += 1
                return ps

            # ---- pass 1: aggregate raw x, then z @ [Wsig|W1] --------------
            with nc.named_scope("pass1"):
                for t in range(NT):
                    ps = seg_gather(t, xrows_d, IN_CH, ps_agg, "agg", True)
                    dv = dinv_sb[:, t:t + 1]
                    xl = xload.tile([TP, IN_CH], bf16, tag="xl",
                                    name=f"xl_{t}")
                    nc.sync.dma_start(
                        out=xl[:], in_=xloc_d.ap()[t * TP:(t + 1) * TP, :])
                    # z = dv * (agg + dv * x_self)
                    zt = epi.tile([TP, IN_CH], f32, tag="zt", name=f"zt_{t}")
                    if ps is not None:
                        nc.vector.scalar_tensor_tensor(
                            zt[:], xl[:], dv, ps[:],
                            mybir.AluOpType.mult, mybir.AluOpType.add)
                    else:
                        nc.vector.scalar_tensor_tensor(
                            zt[:], xl[:], dv, xl[:],
                            mybir.AluOpType.mult, mybir.AluOpType.bypass)
                    zb = epi.tile([TP, IN_CH], bf16, tag="zb", name=f"zb_{t}")
                    nc.scalar.activation(zb[:], zt[:],
                                         mybir.ActivationFunctionType.Copy,
                                         scale=dv)
                    # transpose z and apply the fused weight
                    zT = epi.tile([TP, cfg.KX, TP], bf16, tag="zT",
                                  name=f"zT_{t}")
                    for c in range(cfg.KX):
                        ps_t = ps_sm.tile([TP, TP], bf16, tag="sm",
                                          name=f"ztr_{t}_{c}")
                        nc.tensor.transpose(ps_t[:],
                                            zb[:, c * TP:(c + 1) * TP],
                                            ident_sb[:])
                        nc.vector.tensor_copy(zT[:, c, :], ps_t[:])
                    ps2 = ps_w.tile([TP, FUSED], f32, tag="w")
                    for c in range(cfg.KX):
                        nc.tensor.matmul(ps2[:], zT[:, c, :], wf_sb[:, c, :],
                                         start=(c == 0), stop=(c == cfg.KX - 1))
                    # sig half: relu(. + bsig); mask-sum on PE
                    sig_f = epi.tile([TP, HID], f32, tag="sigf")
                    nc.vector.tensor_tensor(sig_f[:], ps2[:, :HID],
                                            bsig_sb[:], mybir.AluOpType.add)
                    sig_b = epi.tile([TP, HID], bf16, tag="sigb")
                    nc.scalar.activation(sig_b[:], sig_f[:],
                                         mybir.ActivationFunctionType.Relu)
                    nc.tensor.matmul(s_ps[:], mask_sb[:, t:t + 1], sig_b[:],
                                     start=(t == 0), stop=(t == NT - 1))
                    # conv1 half
                    nc.scalar.copy(c1agg_sb[:, t, :], ps2[:, HID:])

            # ---- signature allreduce + gamma/beta -------------------------
            with nc.named_scope("signature"):
                s_sb = one.tile([1, HID], f32)
                nc.scalar.copy(s_sb[:], s_ps[:])
                nc.sync.dma_start(out=sin_d.ap(), in_=s_sb[:])
                nc.gpsimd.collective_compute(
                    "AllReduce", mybir.AluOpType.add, replica_groups=rg,
                    ins=[sin_d.ap().opt()], outs=[sout_d.ap().opt()])

                s_col = one.tile([TP, cfg.KA], f32)
                nc.vector.memset(s_col[:], 0.0)
                nc.vector.memset(s_col[0:1, cfg.KA - 1:cfg.KA], 1.0)
                nc.sync.dma_start(
                    out=s_col[:, 0:2],
                    in_=sout_d.ap().rearrange("o (c p) -> (o c) p", p=TP)
                        .transpose([1, 0]))
                s_rep = one.tile([TP, cfg.KA, TP], f32)
                for c in range(cfg.KA):
                    nc.vector.tensor_copy(
                        s_rep[:, c, :],
                        s_col[:, c:c + 1].to_broadcast((TP, TP)))

                gb_sb = {}
                for nm, width in (("wg1", HID), ("wb1", HID),
                                  ("wg2", OUT), ("wb2", OUT)):
                    ps_fc = ps_sm.tile([TP, width], f32, tag="sm", name=nm)
                    for c in range(cfg.KA):
                        nc.tensor.matmul(ps_fc[:], s_rep[:, c, :],
                                         fc_sb[nm][:, c, :],
                                         start=(c == 0), stop=(c == cfg.KA - 1))
                    gb = one.tile([TP, width], f32, name=f"gb_{nm}", tag=nm)
                    nc.scalar.activation(gb[:], ps_fc[:],
                                         mybir.ActivationFunctionType.Tanh)
                    gb_sb[nm] = gb
                # beta + conv bias
                nc.vector.tensor_tensor(gb_sb["wb1"][:], gb_sb["wb1"][:],
                                        b1c_sb[:], mybir.AluOpType.add)
                nc.vector.tensor_tensor(gb_sb["wb2"][:], gb_sb["wb2"][:],
                                        b2c_sb[:], mybir.AluOpType.add)

            # ---- encoder local: FiLM + relu + LN + conv2 matmul -----------
            def layernorm(dst_ap, src_ap, width):
                st6 = small.tile([TP, 6], f32, tag="st6", name="st6")
                mv = small.tile([TP, 2], f32, tag="mv", name="mv")
                nc.vector.bn_stats(st6[:], src_ap)
                nc.vector.bn_aggr(mv[:], st6[:])
                std = small.tile([TP, 1], f32, tag="std", name="std")
                nc.scalar.activation(std[:], mv[:, 1:2],
                                     mybir.ActivationFunctionType.Sqrt,
                                     bias=eps_sb[:, 0:1])
                rstd = small.tile([TP, 1], f32, tag="rstd", name="rstd")
                nc.vector.reciprocal(rstd[:], std[:])
                nmr = small.tile([TP, 1], f32, tag="nmr", name="nmr")
                nc.vector.scalar_tensor_tensor(
                    nmr[:], mv[:, 0:1], -1.0, rstd[:],
                    mybir.AluOpType.mult, mybir.AluOpType.mult)
                nc.scalar.activation(dst_ap, src_ap,
                                     mybir.ActivationFunctionType.Identity,
                                     bias=nmr[:, 0:1], scale=rstd[:, 0:1])

            with nc.named_scope("encoder_local"):
                # batched FiLM + relu in groups (fewer, larger DVE/Act ops)
                GRP = 8
                hr_sb = persist.tile([TP, NT, HID], bf16)
                for g0 in range(0, NT, GRP):
                    gn = min(GRP, NT - g0)
                    hf_g = epi.tile([TP, GRP, HID], f32, tag="hfg",
                                    name=f"hf_{g0}")
                    nc.vector.tensor_tensor(
                        hf_g[:, :gn, :], c1agg_sb[:, g0:g0 + gn, :],
                        gb_sb["wg1"][:].unsqueeze(1)
                        .to_broadcast((TP, gn, HID)),
                        mybir.AluOpType.mult)
                    nc.vector.tensor_tensor(
                        hf_g[:, :gn, :], hf_g[:, :gn, :],
                        gb_sb["wb1"][:].unsqueeze(1)
                        .to_broadcast((TP, gn, HID)),
                        mybir.AluOpType.add)
                    nc.scalar.activation(hr_sb[:, g0:g0 + gn, :],
                                         hf_g[:, :gn, :],
                                         mybir.ActivationFunctionType.Relu)
                for t in range(NT):
                    h1 = epi.tile([TP, HID], bf16, tag="h1", name=f"h1_{t}")
                    layernorm(h1[:], hr_sb[:, t, :], HID)
                    # transpose h1 tile and matmul with w2
                    h1T = epi.tile([TP, cfg.KH, TP], bf16, tag="h1T",
                                   name=f"h1T_{t}")
                    for c in range(cfg.KH):
                        ps_t = ps_sm.tile([TP, TP], bf16, tag="sm",
                                          name=f"tr_{t}_{c}")
                        nc.tensor.transpose(ps_t[:],
                                            h1[:, c * TP:(c + 1) * TP],
                                            ident_sb[:])
                        nc.vector.tensor_copy(h1T[:, c, :], ps_t[:])
                    ps2 = ps_w.tile([TP, OUT], f32, tag="w", name=f"w2_{t}")
                    for c in range(cfg.KH):
                        nc.tensor.matmul(ps2[:], h1T[:, c, :], w2_sb[:, c, :],
                                         start=(c == 0), stop=(c == cfg.KH - 1))
                    nc.scalar.activation(tp_sb[:, t, :], ps2[:],
                                         mybir.ActivationFunctionType.Copy,
                                         scale=dinv_sb[:, t:t + 1])
                    nc.sync.dma_start(out=tsh_d.ap()[t * TP:(t + 1) * TP, :],
                                      in_=tp_sb[:, t, :])
                nc.gpsimd.collective_compute(
                    "AllGather", mybir.AluOpType.bypass, replica_groups=rg,
                    ins=[tsh_d.ap().opt()], outs=[tfull_d.ap().opt()])

            # ---- pass 2: edge aggregation over t' -------------------------
            with nc.named_scope("pass2"):
                for t in range(NT):
                    ps = seg_gather(t, tfull_d, OUT, ps_agg, "agg", False)
                    dv = dinv_sb[:, t:t + 1]
                    o_f = epi.tile([TP, OUT], f32, tag="of", name=f"o_{t}")
                    if ps is not None:
                        nc.vector.tensor_tensor(o_f[:], ps[:],
                                                tp_sb[:, t, :],
                                                mybir.AluOpType.add)
                    else:
                        nc.vector.tensor_copy(o_f[:], tp_sb[:, t, :])
                    # gamma2 * (dinv * agg) + (beta2 + b2)
                    nc.vector.scalar_tensor_tensor(
                        o_f[:], o_f[:], dv, gb_sb["wg2"][:],
                        mybir.AluOpType.mult, mybir.AluOpType.mult)
                    nc.vector.tensor_tensor(o_f[:], o_f[:], gb_sb["wb2"][:],
                                            mybir.AluOpType.add)
                    o_ln = epi.tile([TP, OUT], f32, tag="oln", name=f"ol_{t}")
                    layernorm(o_ln[:], o_f[:], OUT)
                    nc.sync.dma_start(out=out_d.ap()[t * TP:(t + 1) * TP, :],
                                      in_=o_ln[:])

    nc.compile()
    return nc

# ---------------------------------------------------------------- runner ----


_CACHE = {}


def _get_program(cfg, calls, idx_cols, seg_cols):
    key = (cfg.NT, tuple(calls))
    if key not in _CACHE:
        _CACHE[key] = build_program(cfg, calls, idx_cols, seg_cols)
    return _CACHE[key]


def run(inputs, cfg=FULL, trace=False, **kw):
    in_maps, calls, idx_cols, seg_cols = make_in_maps(inputs, cfg)
    nc = _get_program(cfg, calls, idx_cols, seg_cols)
    res = bass_utils.run_bass_kernel_spmd(
        nc, in_maps, core_ids=list(range(cfg.NC)), trace=trace, **kw)
    out = np.concatenate([res.results[c]["out"] for c in range(cfg.NC)],
                         axis=0)[: cfg.n_real]
    return out.astype(np.float32), res


def kernel(**inputs):
    out, _ = run(inputs, FULL)
    return out
